# revision 1
# baseline (speedup 1.0000x reference)
"""GCN message-passing kernel for Trainium2, 8 NeuronCores (SPMD).

Math (per reference):
    msg[n]  = sum_{e: dst[e]==n} feature[src[e]]
    h[n]    = msg[n] / deg[n]            (0 if deg==0)
    ge      = relu(h @ W_gc + b_gc)      # [N, 3]
    mult[n] = sum_g (ge[n,g] == max_g ge[n,g])
    out     = (h * mult) @ W_lin.T + b_lin

Sharding: nodes are partitioned contiguously across 8 cores by dst
(12500 nodes/core); each core owns every edge pointing into its nodes.
The feature table (25.6 MB, f32) is replicated per core; rows are 256 B,
which is exactly the DGE gather element size, so features stay exact f32
(fp16 was tried and rejected: the argmax-tie multiplier is discontinuous
in h, and fp16 noise flips relu-zero ties on ~30 nodes -> absmax blowup).

Per-core structure (identical program on all 8 cores -> SPMD):
  * src ids are split into 4 ranges of <=32768 rows (gather idx is int16).
  * dst nodes are grouped 256 per PSUM accumulator; each (group, range)
    bucket is split into two 128-node strata so the DVE one-hot is only
    128 wide. Buckets get fixed slot capacities (max over cores/groups,
    rounded to 128-slot columns) so the program is data-independent;
    pad slots gather spread-out rows and carry sentinel -1000 scalars so their
    one-hot column is all zero.
  * For each 128-edge column: one-hot [128e, 128n] via
    vector.tensor_scalar(is_equal) against an iota row (f32 2x mode),
    then TensorE matmul feat.T @ onehot accumulated into PSUM [64, 256]
    -> transposed group sums msgT.  Using msgT as lhsT for both epilogue
    matmuls avoids every transpose.
  * Batched epilogue per superblock (14 windows of 128 nodes):
    ge = relu((msg@W_gc)*invdeg + b_gc); mult = #argmax ties;
    out = (msg@W_lin.T) * (invdeg*mult) + b_lin.

Measured performance notes (axon trn2, via n_reps differencing):
  * dma_gather is per-descriptor bound at ~6.5-7.5 ns/idx (~34 GB/s at
    256 B elements) regardless of single_packet, call size (512-7168),
    or queue count; even fully sequential indices only reach ~45 GB/s,
    so the limit is descriptor processing, not HBM locality.  The
    whole kernel is gather-bound (compute fully overlaps).  Pad idxs
    all pointing at row 0 serialized on HBM (same-address) and cost
    ~0.33 ms; spreading them got 1.34 ms -> ~1.0 ms.  Same-process A/B:
    pads striding a SMALL hot region (1696 rows) beat per-range-wide
    and random spreads (1.76 vs 2.04 vs 2.17 ms/rep that session) --
    hot pads ride open DRAM rows, scattered pads are cold reads.
  * single_packet=True faults the DMA engine above 64 descriptors/
    engine (1024 idxs); single_packet=False is required for big calls.
  * gather idxs live in SBUF partitions 0-15 wrapped i->[i%16, i//16]
    and must be replicated into partitions 16-31 (tx/rx Q7 core pair).
  * Tried and rejected: 512-wide groups without strata (25 groups,
    one-hot width 512, caps (15,14,15,1) = 144k slots, -12% descs)
    measured 2.12 ms vs this design's 1.34 ms in the same process --
    the wider f32 tensor_scalar one-hots + 512-cycle matmul streams
    cost more than the descriptor saving.  Two-tier overflow columns
    only save 1 column (overflow maxima 151-166 > 128), and range
    re-splitting can't beat the binomial tail without host-side node
    rebalancing (means 419.6/bucket vs 384 = 3-column quantum).
"""

import sys

sys.path.insert(0, "/opt/trn_rl_repo")

import numpy as np

from concourse import bacc, bass, mybir, tile
from concourse import bass_utils

P = 128
D = 64
WN = 128  # one-hot width / stratum size (nodes)

N_NODES = 100000
N_CORES = 8
NODES_PER_CORE = N_NODES // N_CORES  # 12500

GW = 256  # dst nodes per group (one PSUM accumulator)
GROUPS = (NODES_PER_CORE + GW - 1) // GW  # 49
NODES_PAD = GROUPS * GW  # 12544
SBG = 7  # groups per superblock
SB = GROUPS // SBG  # 7
assert SBG * SB == GROUPS
WINDOWS = 2 * GROUPS  # 98 windows of 128 nodes
WPB = 2 * SBG  # windows per superblock (epilogue batch) = 14

RANGE = 32768  # int16-addressable gather range
RANGE_STARTS = [0, RANGE, 2 * RANGE, 3 * RANGE]
RANGE_ROWS = [RANGE, RANGE, RANGE, N_NODES - 3 * RANGE]
NR = 4

F32 = mybir.dt.float32
I16 = mybir.dt.int16


def _schedule(caps):
    """Static slot layout from per-range bucket capacities (in columns).

    A bucket is (group, stratum-half, range); capacity caps[r] columns of
    128 slots.  Slot order: (sb, r, j, h, k) -- superblock-major, then
    range (one gather call per (sb, r) segment), then group-in-superblock,
    stratum half, column.
    """
    seg_cols = [SBG * 2 * c for c in caps]  # columns per (sb, r) segment
    sb_cols = sum(seg_cols)
    tot_cols = SB * sb_cols
    seg_off = np.cumsum([0] + seg_cols).tolist()
    return seg_cols, sb_cols, tot_cols, seg_off


def build_program(caps, n_reps=1, parts=("gather", "onehot", "mm", "ep")):
    """Build the single-core Bass program (identical across cores).

    n_reps > 1 repeats the whole compute body (benchmarking aid: device
    time per rep = (T(n) - T(1)) / (n - 1), cancelling dispatch overhead).
    """
    seg_cols, sb_cols, tot_cols, seg_off = _schedule(caps)
    tot_slots = tot_cols * P

    nc = bacc.Bacc("TRN2", target_bir_lowering=False, debug=False)

    feat = nc.dram_tensor("feat32", [N_NODES, D], F32, kind="ExternalInput")
    gidx = nc.dram_tensor("gidx", [P, tot_slots // 16], I16, kind="ExternalInput")
    sc = nc.dram_tensor("sc", [P, tot_cols], F32, kind="ExternalInput")
    invdeg = nc.dram_tensor("invdeg", [P, WINDOWS], F32, kind="ExternalInput")
    wgc = nc.dram_tensor("wgc", [D, 3], F32, kind="ExternalInput")
    wlint = nc.dram_tensor("wlint", [D, D], F32, kind="ExternalInput")
    bgc_rep = nc.dram_tensor("bgc_rep", [P, 3 * WPB], F32, kind="ExternalInput")
    blin_rep = nc.dram_tensor("blin_rep", [P, D * SBG], F32, kind="ExternalInput")
    iota32 = nc.dram_tensor("iota32", [P, WN], F32, kind="ExternalInput")
    out = nc.dram_tensor("out", [NODES_PAD, D], F32, kind="ExternalOutput")

    # output viewed as [p, window, d] so a 7-window batch is one DMA
    out_v = out.ap().rearrange("(w p) d -> p w d", p=P)

    with tile.TileContext(nc) as tc:
        with (
            tc.tile_pool(name="const", bufs=1) as cpool,
            tc.tile_pool(name="seg0", bufs=2) as seg0p,
            tc.tile_pool(name="seg1", bufs=2) as seg1p,
            tc.tile_pool(name="seg2", bufs=2) as seg2p,
            tc.tile_pool(name="seg3", bufs=2) as seg3p,
            tc.tile_pool(name="oh", bufs=4) as ohp,
            tc.tile_pool(name="msg", bufs=SBG + 2) as msgp,
            tc.tile_pool(name="eps", bufs=2) as epsp,
            tc.tile_pool(name="outs", bufs=3) as outsp,
            tc.tile_pool(name="agg", bufs=2, space="PSUM") as aggp,
            tc.tile_pool(name="gep", bufs=2, space="PSUM") as gepp,
            tc.tile_pool(name="zp", bufs=2, space="PSUM") as zpp,
        ):
            segps = [seg0p, seg1p, seg2p, seg3p]

            # ---- preload constants into SBUF ----
            gidx_s = cpool.tile([P, tot_slots // 16], I16, tag="gidx")
            nc.sync.dma_start(out=gidx_s[:], in_=gidx.ap())
            sc_s = cpool.tile([P, tot_cols], F32, tag="sc")
            nc.sync.dma_start(out=sc_s[:], in_=sc.ap())
            inv_s = cpool.tile([P, WINDOWS], F32, tag="invdeg")
            nc.sync.dma_start(out=inv_s[:], in_=invdeg.ap())
            wgc_s = cpool.tile([D, 3], F32, tag="wgc")
            nc.sync.dma_start(out=wgc_s[:], in_=wgc.ap())
            wlt_s = cpool.tile([D, D], F32, tag="wlint")
            nc.sync.dma_start(out=wlt_s[:], in_=wlint.ap())
            bgc_s = cpool.tile([P, 3 * WPB], F32, tag="bgc")
            nc.sync.dma_start(out=bgc_s[:], in_=bgc_rep.ap())
            blin_s = cpool.tile([P, D * SBG], F32, tag="blin")
            nc.sync.dma_start(out=blin_s[:], in_=blin_rep.ap())
            iota_s = cpool.tile([P, WN], F32, tag="iota")
            nc.sync.dma_start(out=iota_s[:], in_=iota32.ap())

            for _rep in range(n_reps):
                slot_off = 0  # running slot offset into gidx
                for sb in range(SB):
                    # ---- gather the superblock's edge features (one call/range)
                    segs = []
                    for r in range(NR):
                        ncols = seg_cols[r]
                        nslots = ncols * P
                        seg = segps[r].tile([P, ncols, D], F32, tag=f"seg{r}")
                        fview = feat.ap()[
                            RANGE_STARTS[r] : RANGE_STARTS[r] + RANGE_ROWS[r], :
                        ]
                        if "gather" not in parts:
                            segs.append(seg)
                            slot_off += nslots
                            continue
                        nc.gpsimd.dma_gather(
                            out_ap=seg[:],
                            in_ap=fview,
                            idxs_ap=gidx_s[:, slot_off // 16 : (slot_off + nslots) // 16],
                            num_idxs=nslots,
                            num_idxs_reg=nslots,
                            elem_size=D,
                            # single-packet coalescing tops out at 64 descriptors
                            # per SDMA engine (1024 idxs); big calls must use
                            # per-descriptor packets or the DMA engine faults
                            single_packet=False,
                        )
                        segs.append(seg)
                        slot_off += nslots

                    # ---- aggregate each group: msgT[64, 256] = feat.T @ onehot
                    msgs = []
                    for j in range(SBG):
                        psum = None
                        if "mm" in parts:
                            psum = aggp.tile([D, GW], F32, tag="agg")
                        # columns: (range, half, k); PSUM start/stop are per
                        # half-region (accumulation groups bind to regions)
                        for h in range(2):
                            cols = []
                            for r in range(NR):
                                for k in range(caps[r]):
                                    segc = (j * 2 + h) * caps[r] + k
                                    cols.append((r, segc))
                            nmm = len(cols)
                            for ci, (r, segc) in enumerate(cols):
                                gcol = sb * sb_cols + seg_off[r] + segc
                                oh = None
                                if "onehot" in parts:
                                    oh = ohp.tile([P, WN], F32, tag="oh")
                                if "onehot" in parts:
                                    nc.vector.tensor_scalar(
                                    out=oh[:],
                                    in0=iota_s[:],
                                    scalar1=sc_s[:, gcol : gcol + 1],
                                    scalar2=None,
                                        op0=mybir.AluOpType.is_equal,
                                    )
                                if "mm" in parts:
                                    nc.tensor.matmul(
                                    out=psum[:, h * WN : (h + 1) * WN],
                                    lhsT=segs[r][:, segc, :],
                                    rhs=(oh[:] if oh is not None else iota_s[:]),
                                        start=(ci == 0),
                                        stop=(ci == nmm - 1),
                                    )
                        if "mm" in parts:
                            msgT = msgp.tile([D, GW], F32, tag="msg")
                            nc.scalar.copy(out=msgT[:], in_=psum[:])
                            msgs.append(msgT)

                    if "ep" not in parts:
                        continue
                    # ---- epilogue over this superblock's 14 windows ----
                    inv_sb = inv_s[:, sb * WPB : (sb + 1) * WPB]  # [128, 14]
                    inv_b = inv_sb.rearrange("p (w o) -> p w o", o=1).to_broadcast(
                        [P, WPB, 3]
                    )
                    gp = gepp.tile([P, 3 * WPB], F32, tag="gep")
                    for w in range(WPB):
                        j, wi = w // 2, w % 2
                        nc.tensor.matmul(
                            out=gp[:, 3 * w : 3 * w + 3],
                            lhsT=msgs[j][:, wi * P : (wi + 1) * P],
                            rhs=wgc_s[:],
                            start=True,
                            stop=True,
                        )
                    ge_s = epsp.tile([P, 3 * WPB], F32, tag="ge")
                    gp3 = gp[:].rearrange("p (w g) -> p w g", g=3)
                    ge3 = ge_s[:].rearrange("p (w g) -> p w g", g=3)
                    nc.vector.tensor_tensor(
                        out=ge3, in0=gp3, in1=inv_b, op=mybir.AluOpType.mult
                    )
                    nc.vector.tensor_tensor(
                        out=ge_s[:], in0=ge_s[:], in1=bgc_s[:], op=mybir.AluOpType.add
                    )
                    nc.vector.tensor_scalar(
                        out=ge_s[:],
                        in0=ge_s[:],
                        scalar1=0.0,
                        scalar2=None,
                        op0=mybir.AluOpType.max,
                    )
                    top = epsp.tile([P, WPB], F32, tag="top")
                    nc.vector.tensor_reduce(
                        out=top[:],
                        in_=ge3,
                        axis=mybir.AxisListType.X,
                        op=mybir.AluOpType.max,
                    )
                    mask = epsp.tile([P, 3 * WPB], F32, tag="mask")
                    top_b = top[:].rearrange("p (w o) -> p w o", o=1).to_broadcast(
                        [P, WPB, 3]
                    )
                    nc.vector.tensor_tensor(
                        out=mask[:].rearrange("p (w g) -> p w g", g=3),
                        in0=ge3,
                        in1=top_b,
                        op=mybir.AluOpType.is_equal,
                    )
                    mult_t = epsp.tile([P, WPB], F32, tag="mult")
                    nc.vector.tensor_reduce(
                        out=mult_t[:],
                        in_=mask[:].rearrange("p (w g) -> p w g", g=3),
                        axis=mybir.AxisListType.X,
                        op=mybir.AluOpType.add,
                    )
                    q = epsp.tile([P, WPB], F32, tag="q")
                    nc.vector.tensor_tensor(
                        out=q[:], in0=mult_t[:], in1=inv_sb, op=mybir.AluOpType.mult
                    )

                    for half in range(2):
                        zp = zpp.tile([P, D * SBG], F32, tag="zp")
                        for k in range(SBG):
                            w = half * SBG + k
                            j, wi = w // 2, w % 2
                            nc.tensor.matmul(
                                out=zp[:, k * D : (k + 1) * D],
                                lhsT=msgs[j][:, wi * P : (wi + 1) * P],
                                rhs=wlt_s[:],
                                start=True,
                                stop=True,
                            )
                        os_ = outsp.tile([P, D * SBG], F32, tag="outs")
                        qh = (
                            q[:, half * SBG : (half + 1) * SBG]
                            .rearrange("p (w o) -> p w o", o=1)
                            .to_broadcast([P, SBG, D])
                        )
                        nc.vector.tensor_tensor(
                            out=os_[:].rearrange("p (w d) -> p w d", d=D),
                            in0=zp[:].rearrange("p (w d) -> p w d", d=D),
                            in1=qh,
                            op=mybir.AluOpType.mult,
                        )
                        nc.vector.tensor_tensor(
                            out=os_[:], in0=os_[:], in1=blin_s[:], op=mybir.AluOpType.add
                        )
                        w0 = sb * WPB + half * SBG
                        nc.sync.dma_start(
                            out=out_v[:, w0 : w0 + SBG, :],
                            in_=os_[:].rearrange("p (w d) -> p w d", d=D),
                        )

    nc.compile()
    return nc


def host_prep(feature, src, dst, W_gc, b_gc, W_lin, b_lin):
    """Shard + lay out per-core inputs. Returns (in_maps, caps)."""
    src = np.asarray(src).astype(np.int64)
    dst = np.asarray(dst).astype(np.int64)
    feat32 = np.ascontiguousarray(np.asarray(feature, dtype=np.float32))

    core_of = dst // NODES_PER_CORE
    nloc_all = dst - core_of * NODES_PER_CORE

    deg = np.bincount(dst, minlength=N_NODES).astype(np.float32)
    invd = np.where(deg > 0, 1.0 / np.maximum(deg, 1.0), 0.0).astype(np.float32)

    # bucket = (group, stratum-half, range)
    r_all = (src // RANGE).astype(np.int64)
    grp_all = nloc_all // GW
    half_all = (nloc_all % GW) // WN
    NB = GROUPS * 2 * NR  # buckets per core
    max_cnt = np.zeros(NR, dtype=np.int64)
    per_core = []
    for c in range(N_CORES):
        m = core_of == c
        e_src = src[m]
        e_nloc = nloc_all[m]
        e_r = r_all[m]
        e_grp = grp_all[m]
        e_half = half_all[m]
        key = (e_grp * 2 + e_half) * NR + e_r
        cnt = np.bincount(key, minlength=NB).reshape(GROUPS * 2, NR)
        max_cnt = np.maximum(max_cnt, cnt.max(axis=0))
        per_core.append((e_src, e_nloc, e_r, e_grp, e_half, key))

    caps = [max(1, int(-(-int(mc) // P))) for mc in max_cnt]  # cols per bucket
    seg_cols, sb_cols, tot_cols, seg_off = _schedule(caps)
    tot_slots = tot_cols * P

    # constants (shared across cores)
    iota32 = np.broadcast_to(np.arange(WN, dtype=np.float32), (P, WN)).copy()
    wgc = np.ascontiguousarray(np.asarray(W_gc, dtype=np.float32))
    wlint = np.ascontiguousarray(np.asarray(W_lin, dtype=np.float32).T)
    bgc_rep = np.tile(np.asarray(b_gc, dtype=np.float32).reshape(1, 3), (P, WPB))
    blin_rep = np.tile(np.asarray(b_lin, dtype=np.float32).reshape(1, D), (P, SBG))

    capv = np.array(caps, dtype=np.int64)
    segoffv = np.array(seg_off[:NR], dtype=np.int64)

    in_maps = []
    for c in range(N_CORES):
        e_src, e_nloc, e_r, e_grp, e_half, key = per_core[c]

        order = np.argsort(key, kind="stable")
        k_sorted = key[order]
        start_of = np.zeros(NB, dtype=np.int64)
        start_of[1:] = np.cumsum(np.bincount(k_sorted, minlength=NB))[:-1]
        rank = np.arange(k_sorted.size) - start_of[k_sorted]

        g_s = e_grp[order]
        h_s = e_half[order]
        r_s = e_r[order]
        sb_s = g_s // SBG
        j_s = g_s % SBG
        assert (rank < capv[r_s] * P).all(), "bucket capacity exceeded"

        slot = (
            sb_s * (sb_cols * P)
            + segoffv[r_s] * P
            + (j_s * 2 + h_s) * capv[r_s] * P
            + rank
        )

        # pad slots must hold valid in-range rows.  All-row-0 pads
        # serialize on HBM (const-idx gathers: 9 vs 34 GB/s); spreading
        # over a SMALL hot region (1696 rows, valid for every range)
        # measured faster than spreading over each range's full 32k rows
        # (1.76 vs 2.04 ms/rep same-process) -- pads stay in open DRAM
        # rows instead of being cold random reads.
        pad_rows = min(RANGE_ROWS)
        gidx_flat = ((np.arange(tot_slots, dtype=np.int64) * 7) % pad_rows).astype(
            np.int16
        )
        gidx_flat[slot] = (e_src[order] - r_s * RANGE).astype(np.int16)
        scv = np.full(tot_slots, -1000.0, dtype=np.float32)
        scv[slot] = (e_nloc[order] - (g_s * 2 + h_s) * WN).astype(np.float32)

        # gather idx wrapping: idx i -> [i%16, i//16], replicated into
        # partitions 16-31 for the Q7 tx/rx core pair; remaining rows must
        # still hold valid (>= -1, in-range) values for the sim's checks
        gidx_w = np.zeros((P, tot_slots // 16), dtype=np.int16)
        wrapped = gidx_flat.reshape(-1, 16).T
        gidx_w[:16] = wrapped
        gidx_w[16:32] = wrapped
        # sc: slot -> (partition slot%128, col slot//128)
        sc_arr = np.ascontiguousarray(scv.reshape(-1, P).T)

        iv = np.zeros(NODES_PAD, dtype=np.float32)
        iv[:NODES_PER_CORE] = invd[c * NODES_PER_CORE : (c + 1) * NODES_PER_CORE]
        invdeg_c = np.ascontiguousarray(iv.reshape(WINDOWS, P).T)

        in_maps.append(
            {
                "feat32": feat32,
                "gidx": gidx_w,
                "sc": sc_arr,
                "invdeg": invdeg_c,
                "wgc": wgc,
                "wlint": wlint,
                "bgc_rep": bgc_rep,
                "blin_rep": blin_rep,
                "iota32": iota32,
            }
        )

    return in_maps, tuple(caps)


_PROGRAM_CACHE = {}


def kernel(**inputs):
    feature = inputs["feature"]
    src = inputs["src"]
    dst = inputs["dst"]
    in_maps, caps = host_prep(
        feature,
        src,
        dst,
        inputs["W_gc"],
        inputs["b_gc"],
        inputs["W_lin"],
        inputs["b_lin"],
    )
    if caps not in _PROGRAM_CACHE:
        _PROGRAM_CACHE[caps] = build_program(caps)
    nc = _PROGRAM_CACHE[caps]
    res = bass_utils.run_bass_kernel_spmd(nc, in_maps, core_ids=list(range(N_CORES)))
    out = np.concatenate(
        [res.results[c]["out"][:NODES_PER_CORE] for c in range(N_CORES)], axis=0
    )
    return out.astype(np.float32)



# revision 4
# speedup vs baseline: 71.0168x; 71.0168x over previous
"""GCN message-passing kernel for Trainium2, 8 NeuronCores (SPMD).

Math (per reference):
    msg[n]  = sum_{e: dst[e]==n} feature[src[e]]
    h[n]    = msg[n] / deg[n]            (0 if deg==0)
    ge      = relu(h @ W_gc + b_gc)      # [N, 3]
    mult[n] = sum_g (ge[n,g] == max_g ge[n,g])
    out     = (h * mult) @ W_lin.T + b_lin

Sharding: nodes partitioned contiguously across 8 cores by dst (12500
nodes/core); each core owns every edge into its nodes.  The f32 feature
table is replicated per core (rows are 256 B = the DGE gather element).

v2 changes over the 1.27 ms baseline (all measured on axon trn2 via NTFF
profiles; baseline was Vector-bound 92% / GpSimd 81%):
  * Gathers round-robin over 4 SWDGE queues (num_swdge_queues=4).  Queue 0
    desc-gen runs on the GpSimd engine (~7.8 ns/desc); queues 1-3 run on
    other Q7 core pairs concurrently (~16.4 ns/desc each).  4-queue
    round-robin sustains ~4.1 ns/desc vs ~7.9 single-queue.  Gather idxs
    must be replicated into ALL EIGHT 16-partition windows (2-window
    replication corrupts queues 1-3 - verified on HW).
  * One-hots built with ONE batched tensor_tensor is_equal per (group,
    range) block: in0 = sc column broadcast along W (stride-0), in1 =
    replicated-iota tile.  ~136 ns/column vs 1153 ns/column for the old
    per-column tensor_scalar with a per-partition scalar AP (the scalar
    AP load dominates; immediate scalars and TT broadcasts are fast).
  * Per-bucket slot capacities: each (group, half, range) bucket gets
    ceil128(max-over-cores count) of its own instead of a global
    per-range cap - cuts padded gather slots ~15%.
  * fp32r was evaluated and rejected: walrus requires operands pre-rounded
    to fp32r (reduced precision), same tie-flip risk as the rejected fp16.

Structure: dst nodes grouped 256 per PSUM accumulator, split in two
128-node strata so the one-hot is 128 wide; src ids split into 4 ranges
of <=32768 rows (gather idx is int16, hard API limit).  Bucket =
(group, half, range).  Pad slots gather spread-out rows of a small hot
region and carry sentinel -1000 so their one-hot column is all zero.
"""

import sys

sys.path.insert(0, "/opt/trn_rl_repo")

import numpy as np

from concourse import bacc, bass, mybir, tile
from concourse import bass_utils

P = 128
D = 64
WN = 128  # one-hot width / stratum size (nodes)

N_NODES = 100000
N_CORES = 8
NODES_PER_CORE = N_NODES // N_CORES  # 12500

GW = 256  # dst nodes per group (one PSUM accumulator)
GROUPS = (NODES_PER_CORE + GW - 1) // GW  # 49
NODES_PAD = GROUPS * GW  # 12544
SBG = 7  # groups per superblock
SB = GROUPS // SBG  # 7
assert SBG * SB == GROUPS
WINDOWS = 2 * GROUPS  # 98 windows of 128 nodes
WPB = 2 * SBG  # windows per superblock (epilogue batch) = 14

RANGE = 32768  # int16-addressable gather range
RANGE_STARTS = [0, RANGE, 2 * RANGE, 3 * RANGE]
RANGE_ROWS = [RANGE, RANGE, RANGE, N_NODES - 3 * RANGE]
NR = 4

# measured effective per-descriptor rates for weighted queue assignment
QUEUE_RATE = [7.84, 16.4, 16.4, 16.4]

F32 = mybir.dt.float32
I16 = mybir.dt.int16


def _layout(caps):
    """Static slot/column layout from per-bucket capacities.

    caps: int array [GROUPS, 2, NR] - columns (of 128 slots) per bucket.
    Column order: sb-major, then range (one gather call per (sb, r)
    segment), then group-in-superblock, then half, then column.
    Returns (col_base, seg_cols, seg_col_off, tot_cols):
      col_base[jg, h, r]: global column index of bucket start
      seg_cols[sb][r]: columns in that gather segment
      seg_col_off[sb][r]: global column offset of that segment
    """
    caps = np.asarray(caps).reshape(GROUPS, 2, NR)
    col_base = np.zeros((GROUPS, 2, NR), dtype=np.int64)
    seg_cols = [[0] * NR for _ in range(SB)]
    seg_col_off = [[0] * NR for _ in range(SB)]
    col = 0
    for sb in range(SB):
        for r in range(NR):
            seg_col_off[sb][r] = col
            for j in range(SBG):
                jg = sb * SBG + j
                for h in range(2):
                    col_base[jg, h, r] = col
                    col += int(caps[jg, h, r])
            seg_cols[sb][r] = col - seg_col_off[sb][r]
    return col_base, seg_cols, seg_col_off, col


def build_program(caps, n_reps=1):
    caps = np.asarray(caps).reshape(GROUPS, 2, NR)
    col_base, seg_cols, seg_col_off, tot_cols = _layout(caps)
    tot_slots = tot_cols * P
    # max batched one-hot width per (group, range): both halves
    max_k = int((caps.sum(axis=1)).max())

    # weighted least-loaded queue assignment per (sb, r) gather call
    qload = [0.0, 0.0, 0.0, 0.0]
    qassign = {}
    for sb in range(SB):
        for r in range(NR):
            nd = seg_cols[sb][r] * P
            q = min(range(4), key=lambda i: qload[i] + nd * QUEUE_RATE[i])
            qassign[(sb, r)] = q
            qload[q] += nd * QUEUE_RATE[q]

    nc = bacc.Bacc("TRN2", target_bir_lowering=False, debug=False,
                   num_swdge_queues=4)

    feat = nc.dram_tensor("feat32", [N_NODES, D], F32, kind="ExternalInput")
    gidx = nc.dram_tensor("gidx", [P, tot_slots // 16], I16, kind="ExternalInput")
    sc = nc.dram_tensor("sc", [P, tot_cols], F32, kind="ExternalInput")
    invdeg = nc.dram_tensor("invdeg", [P, WINDOWS], F32, kind="ExternalInput")
    wgc = nc.dram_tensor("wgc", [D, 3], F32, kind="ExternalInput")
    wlint = nc.dram_tensor("wlint", [D, D], F32, kind="ExternalInput")
    bgc_rep = nc.dram_tensor("bgc_rep", [P, 3 * WPB], F32, kind="ExternalInput")
    blin_rep = nc.dram_tensor("blin_rep", [P, D * SBG], F32, kind="ExternalInput")
    repiota = nc.dram_tensor("repiota", [P, max_k * WN], F32, kind="ExternalInput")
    out = nc.dram_tensor("out", [NODES_PAD, D], F32, kind="ExternalOutput")

    out_v = out.ap().rearrange("(w p) d -> p w d", p=P)

    with tile.TileContext(nc) as tc:
        with (
            tc.tile_pool(name="const", bufs=1) as cpool,
            tc.tile_pool(name="seg0", bufs=2) as seg0p,
            tc.tile_pool(name="seg1", bufs=2) as seg1p,
            tc.tile_pool(name="seg2", bufs=2) as seg2p,
            tc.tile_pool(name="seg3", bufs=2) as seg3p,
            tc.tile_pool(name="oh", bufs=8) as ohp,
            tc.tile_pool(name="msg", bufs=SBG + 2) as msgp,
            tc.tile_pool(name="eps", bufs=2) as epsp,
            tc.tile_pool(name="outs", bufs=3) as outsp,
            tc.tile_pool(name="agg", bufs=2, space="PSUM") as aggp,
            tc.tile_pool(name="gep", bufs=2, space="PSUM") as gepp,
            tc.tile_pool(name="zp", bufs=2, space="PSUM") as zpp,
        ):
            segps = [seg0p, seg1p, seg2p, seg3p]

            gidx_s = cpool.tile([P, tot_slots // 16], I16, tag="gidx")
            nc.sync.dma_start(out=gidx_s[:], in_=gidx.ap())
            sc_s = cpool.tile([P, tot_cols], F32, tag="sc")
            nc.sync.dma_start(out=sc_s[:], in_=sc.ap())
            inv_s = cpool.tile([P, WINDOWS], F32, tag="invdeg")
            nc.sync.dma_start(out=inv_s[:], in_=invdeg.ap())
            wgc_s = cpool.tile([D, 3], F32, tag="wgc")
            nc.sync.dma_start(out=wgc_s[:], in_=wgc.ap())
            wlt_s = cpool.tile([D, D], F32, tag="wlint")
            nc.sync.dma_start(out=wlt_s[:], in_=wlint.ap())
            bgc_s = cpool.tile([P, 3 * WPB], F32, tag="bgc")
            nc.sync.dma_start(out=bgc_s[:], in_=bgc_rep.ap())
            blin_s = cpool.tile([P, D * SBG], F32, tag="blin")
            nc.sync.dma_start(out=blin_s[:], in_=blin_rep.ap())
            ri_s = cpool.tile([P, max_k * WN], F32, tag="repiota")
            nc.sync.dma_start(out=ri_s[:], in_=repiota.ap())

            for _rep in range(n_reps):
                for sb in range(SB):
                    # ---- gather this superblock's edge features ----
                    segs = []
                    for r in range(NR):
                        ncols = seg_cols[sb][r]
                        nslots = ncols * P
                        slot_off = seg_col_off[sb][r] * P
                        seg = segps[r].tile([P, ncols, D], F32, tag=f"seg{r}")
                        nc.gpsimd.dma_gather(
                            out_ap=seg[:],
                            in_ap=feat.ap()[
                                RANGE_STARTS[r] : RANGE_STARTS[r] + RANGE_ROWS[r], :
                            ],
                            idxs_ap=gidx_s[:, slot_off // 16 : (slot_off + nslots) // 16],
                            num_idxs=nslots,
                            num_idxs_reg=nslots,
                            elem_size=D,
                            single_packet=False,
                            queue_num=qassign[(sb, r)],
                        )
                        segs.append(seg)

                    # ---- aggregate each group: msgT[64, 256] ----
                    msgs = []
                    for j in range(SBG):
                        jg = sb * SBG + j
                        # batched one-hots: one TT per (group, range)
                        ohs = []
                        for r in range(NR):
                            c0 = int(caps[jg, 0, r])
                            c1 = int(caps[jg, 1, r])
                            k = c0 + c1
                            gcol = int(col_base[jg, 0, r])
                            oh = ohp.tile([P, max_k * WN], F32, tag="oh")
                            scb = (
                                sc_s[:, gcol : gcol + k]
                                .rearrange("p (k o) -> p k o", o=1)
                                .to_broadcast([P, k, WN])
                            )
                            nc.vector.tensor_tensor(
                                out=oh[:, : k * WN].rearrange(
                                    "p (k w) -> p k w", w=WN
                                ),
                                in0=scb,
                                in1=ri_s[:, : k * WN].rearrange(
                                    "p (k w) -> p k w", w=WN
                                ),
                                op=mybir.AluOpType.is_equal,
                            )
                            ohs.append(oh)

                        psum = aggp.tile([D, GW], F32, tag="agg")
                        for h in range(2):
                            cols = []
                            for r in range(NR):
                                c0 = int(caps[jg, 0, r])
                                ch = int(caps[jg, h, r])
                                seg_base = int(col_base[jg, 0, r]) - seg_col_off[sb][r]
                                oh_off = c0 if h == 1 else 0
                                for k in range(ch):
                                    cols.append((r, seg_base + oh_off + k, oh_off + k))
                            nmm = len(cols)
                            for ci, (r, segc, ohc) in enumerate(cols):
                                nc.tensor.matmul(
                                    out=psum[:, h * WN : (h + 1) * WN],
                                    lhsT=segs[r][:, segc, :],
                                    rhs=ohs[r][:, ohc * WN : (ohc + 1) * WN],
                                    start=(ci == 0),
                                    stop=(ci == nmm - 1),
                                )
                        msgT = msgp.tile([D, GW], F32, tag="msg")
                        nc.scalar.copy(out=msgT[:], in_=psum[:])
                        msgs.append(msgT)

                    # ---- epilogue over this superblock's 14 windows ----
                    inv_sb = inv_s[:, sb * WPB : (sb + 1) * WPB]  # [128, 14]
                    inv_b = inv_sb.rearrange("p (w o) -> p w o", o=1).to_broadcast(
                        [P, WPB, 3]
                    )
                    gp = gepp.tile([P, 3 * WPB], F32, tag="gep")
                    for w in range(WPB):
                        j, wi = w // 2, w % 2
                        nc.tensor.matmul(
                            out=gp[:, 3 * w : 3 * w + 3],
                            lhsT=msgs[j][:, wi * P : (wi + 1) * P],
                            rhs=wgc_s[:],
                            start=True,
                            stop=True,
                        )
                    ge_s = epsp.tile([P, 3 * WPB], F32, tag="ge")
                    gp3 = gp[:].rearrange("p (w g) -> p w g", g=3)
                    ge3 = ge_s[:].rearrange("p (w g) -> p w g", g=3)
                    nc.vector.tensor_tensor(
                        out=ge3, in0=gp3, in1=inv_b, op=mybir.AluOpType.mult
                    )
                    nc.vector.tensor_tensor(
                        out=ge_s[:], in0=ge_s[:], in1=bgc_s[:], op=mybir.AluOpType.add
                    )
                    nc.vector.tensor_scalar(
                        out=ge_s[:],
                        in0=ge_s[:],
                        scalar1=0.0,
                        scalar2=None,
                        op0=mybir.AluOpType.max,
                    )
                    top = epsp.tile([P, WPB], F32, tag="top")
                    nc.vector.tensor_reduce(
                        out=top[:],
                        in_=ge3,
                        axis=mybir.AxisListType.X,
                        op=mybir.AluOpType.max,
                    )
                    mask = epsp.tile([P, 3 * WPB], F32, tag="mask")
                    top_b = top[:].rearrange("p (w o) -> p w o", o=1).to_broadcast(
                        [P, WPB, 3]
                    )
                    nc.vector.tensor_tensor(
                        out=mask[:].rearrange("p (w g) -> p w g", g=3),
                        in0=ge3,
                        in1=top_b,
                        op=mybir.AluOpType.is_equal,
                    )
                    mult_t = epsp.tile([P, WPB], F32, tag="mult")
                    nc.vector.tensor_reduce(
                        out=mult_t[:],
                        in_=mask[:].rearrange("p (w g) -> p w g", g=3),
                        axis=mybir.AxisListType.X,
                        op=mybir.AluOpType.add,
                    )
                    q = epsp.tile([P, WPB], F32, tag="q")
                    nc.vector.tensor_tensor(
                        out=q[:], in0=mult_t[:], in1=inv_sb, op=mybir.AluOpType.mult
                    )

                    for half in range(2):
                        zp = zpp.tile([P, D * SBG], F32, tag="zp")
                        for k in range(SBG):
                            w = half * SBG + k
                            j, wi = w // 2, w % 2
                            nc.tensor.matmul(
                                out=zp[:, k * D : (k + 1) * D],
                                lhsT=msgs[j][:, wi * P : (wi + 1) * P],
                                rhs=wlt_s[:],
                                start=True,
                                stop=True,
                            )
                        os_ = outsp.tile([P, D * SBG], F32, tag="outs")
                        qh = (
                            q[:, half * SBG : (half + 1) * SBG]
                            .rearrange("p (w o) -> p w o", o=1)
                            .to_broadcast([P, SBG, D])
                        )
                        nc.vector.tensor_tensor(
                            out=os_[:].rearrange("p (w d) -> p w d", d=D),
                            in0=zp[:].rearrange("p (w d) -> p w d", d=D),
                            in1=qh,
                            op=mybir.AluOpType.mult,
                        )
                        nc.vector.tensor_tensor(
                            out=os_[:], in0=os_[:], in1=blin_s[:], op=mybir.AluOpType.add
                        )
                        w0 = sb * WPB + half * SBG
                        nc.sync.dma_start(
                            out=out_v[:, w0 : w0 + SBG, :],
                            in_=os_[:].rearrange("p (w d) -> p w d", d=D),
                        )

    nc.compile()
    return nc


def host_prep(feature, src, dst, W_gc, b_gc, W_lin, b_lin):
    """Shard + lay out per-core inputs. Returns (in_maps, caps_key)."""
    src = np.asarray(src).astype(np.int64)
    dst = np.asarray(dst).astype(np.int64)
    feat32 = np.ascontiguousarray(np.asarray(feature, dtype=np.float32))

    core_of = dst // NODES_PER_CORE
    nloc_all = dst - core_of * NODES_PER_CORE

    deg = np.bincount(dst, minlength=N_NODES).astype(np.float32)
    invd = np.where(deg > 0, 1.0 / np.maximum(deg, 1.0), 0.0).astype(np.float32)

    # bucket = (group, stratum-half, range)
    r_all = (src // RANGE).astype(np.int64)
    grp_all = nloc_all // GW
    half_all = (nloc_all % GW) // WN
    NB = GROUPS * 2 * NR  # buckets per core
    cnt_max = np.zeros(NB, dtype=np.int64)
    per_core = []
    for c in range(N_CORES):
        m = core_of == c
        e_src = src[m]
        e_nloc = nloc_all[m]
        e_r = r_all[m]
        e_grp = grp_all[m]
        e_half = half_all[m]
        key = (e_grp * 2 + e_half) * NR + e_r
        cnt = np.bincount(key, minlength=NB)
        cnt_max = np.maximum(cnt_max, cnt)
        per_core.append((e_src, e_nloc, e_r, e_grp, e_half, key))

    # per-bucket capacities in 128-slot columns (>=1 col per bucket)
    caps = np.maximum(1, -(-cnt_max // P)).reshape(GROUPS, 2, NR)
    col_base, seg_cols, seg_col_off, tot_cols = _layout(caps)
    tot_slots = tot_cols * P
    max_k = int((caps.sum(axis=1)).max())

    # col_base flattened per bucket key for slot assignment
    bucket_base = np.zeros(NB, dtype=np.int64)
    for jg in range(GROUPS):
        for h in range(2):
            for r in range(NR):
                bucket_base[(jg * 2 + h) * NR + r] = col_base[jg, h, r] * P

    # constants (shared across cores)
    repiota = np.tile(np.arange(WN, dtype=np.float32), (P, max_k)).reshape(
        P, max_k * WN
    )
    wgc = np.ascontiguousarray(np.asarray(W_gc, dtype=np.float32))
    wlint = np.ascontiguousarray(np.asarray(W_lin, dtype=np.float32).T)
    bgc_rep = np.tile(np.asarray(b_gc, dtype=np.float32).reshape(1, 3), (P, WPB))
    blin_rep = np.tile(np.asarray(b_lin, dtype=np.float32).reshape(1, D), (P, SBG))

    in_maps = []
    for c in range(N_CORES):
        e_src, e_nloc, e_r, e_grp, e_half, key = per_core[c]

        order = np.argsort(key, kind="stable")
        k_sorted = key[order]
        start_of = np.zeros(NB, dtype=np.int64)
        start_of[1:] = np.cumsum(np.bincount(k_sorted, minlength=NB))[:-1]
        rank = np.arange(k_sorted.size) - start_of[k_sorted]

        slot = bucket_base[k_sorted] + rank
        assert (rank < caps.reshape(-1)[k_sorted] * P).all(), "bucket cap exceeded"

        # pad slots gather spread-out rows of a small hot region (valid for
        # every range); all-same-row pads serialize on HBM.
        pad_rows = min(RANGE_ROWS)
        gidx_flat = ((np.arange(tot_slots, dtype=np.int64) * 7) % pad_rows).astype(
            np.int16
        )
        gidx_flat[slot] = (e_src[order] - e_r[order] * RANGE).astype(np.int16)
        scv = np.full(tot_slots, -1000.0, dtype=np.float32)
        scv[slot] = (e_nloc[order] % WN).astype(np.float32)

        # wrap idx i -> [i%16, i//16]; replicate into ALL 8 16-partition
        # windows (required for 4-queue SWDGE: each queue's Q7 core pair
        # reads its own window)
        wrapped = gidx_flat.reshape(-1, 16).T
        gidx_w = np.tile(wrapped, (8, 1)).astype(np.int16)
        sc_arr = np.ascontiguousarray(scv.reshape(-1, P).T)

        iv = np.zeros(NODES_PAD, dtype=np.float32)
        iv[:NODES_PER_CORE] = invd[c * NODES_PER_CORE : (c + 1) * NODES_PER_CORE]
        invdeg_c = np.ascontiguousarray(iv.reshape(WINDOWS, P).T)

        in_maps.append(
            {
                "feat32": feat32,
                "gidx": gidx_w,
                "sc": sc_arr,
                "invdeg": invdeg_c,
                "wgc": wgc,
                "wlint": wlint,
                "bgc_rep": bgc_rep,
                "blin_rep": blin_rep,
                "repiota": repiota,
            }
        )

    return in_maps, tuple(caps.reshape(-1).tolist())


_PROGRAM_CACHE = {}


def kernel(**inputs):
    feature = inputs["feature"]
    src = inputs["src"]
    dst = inputs["dst"]
    in_maps, caps = host_prep(
        feature,
        src,
        dst,
        inputs["W_gc"],
        inputs["b_gc"],
        inputs["W_lin"],
        inputs["b_lin"],
    )
    if caps not in _PROGRAM_CACHE:
        _PROGRAM_CACHE[caps] = build_program(caps)
    nc = _PROGRAM_CACHE[caps]
    res = bass_utils.run_bass_kernel_spmd(nc, in_maps, core_ids=list(range(N_CORES)))
    out = np.concatenate(
        [res.results[c]["out"][:NODES_PER_CORE] for c in range(N_CORES)], axis=0
    )
    return out.astype(np.float32)
